# revision 30
# baseline (speedup 1.0000x reference)
"""Trainium2 Bass kernel for nn_F0Collisions: batched Chang-Cooper implicit
Fokker-Planck solve, 16384 x 512, data-parallel over rows across 8 cores.

Method (v3.1): each row's tridiagonal system depends on the row only through
one scalar lam (the 3-step beta fixed point collapses to beta = 1/T_f on
this grid).  The host computes lam per row exactly (float64 moments -- the
same moments it must compute anyway to calibrate the Chebyshev interval)
and ships, per 128-row tile, the transposed 2-term-bf16-split Chebyshev
basis lhsT [24, 128].  The Thomas solve is re-gauged ("q-gauge") so the
forward scan consumes f directly (no premultiply):

    A_j  = alpha_j * betac_{j-1}/betac_j      (A_0 = 0)
    q_j  = A_j q_{j-1} + f_j                  (forward scan)
    Cp_j = cp_j * betac_{j+1}/betac_j         (Cp_511 = 0)
    r_j  = q_j - Cp_j r_{j+1}                 (backward scan)
    x_j  = betac_j * r_j                      (one final multiply)

Device work per 128x512 tile: 3 PE matmuls interpolate the A / -Cp / betac
profiles into PSUM; VectorE runs the two scans reading the tables straight
from PSUM (1-port SBUF -> immune to Pool port contention); ScalarE copies
betac PSUM->SBUF; the Pool engine does the final elementwise multiply.
Tiles are processed in PAIRS: since A_0 = 0 and Cp_511 = 0 exactly (table
columns are identically zero), the scan carry self-clears at tile
boundaries, so each scan instruction covers two tiles [128, 1024] with the
pair's tables adjacent in a 2-bank PSUM tile (bwd pair halves swapped to
match the reversed stream).  VectorE does nothing but the 16 scans -- the
roofline for this kernel.  The last pair's final multiply runs on VectorE
after its scans (shortest tail), all others on Pool.
"""

import numpy as np
import ml_dtypes

import concourse.bass as bass
import concourse.mybir as mybir
import concourse.tile as tile
from concourse import bacc
from concourse.bass_utils import run_bass_kernel_spmd

NX, NV = 16384, 512
N_CORES = 8
ROWS = NX // N_CORES          # rows per core
NT = ROWS // 128              # 128-row tiles per core
NP = NT // 2                  # tile pairs
DV = 8.0 / NV
NUEE_COEFF = 2.221e-7
M = 8                         # Chebyshev terms
KS = 3 * M                    # [Ph; Ph; Pl] x [Kh; Kl; Kh] split contraction

F32 = mybir.dt.float32
BF16 = mybir.dt.bfloat16
ALU = mybir.AluOpType
AFT = mybir.ActivationFunctionType


# ---------------------------------------------------------------- host math

def _host_weights(v):
    """v2 and g weight vectors (float64): S2 = sum f*v2, Sg = sum f*g."""
    v = v.astype(np.float64)
    v2 = v * v
    we = (0.5 * (v[1:] + v[:-1])) ** 2 * DV / np.sqrt(2.0)   # sqrt_eps * d_eps
    g = np.empty(NV)
    g[0] = 0.5 * we[0]
    g[-1] = 0.5 * we[-1]
    g[1:-1] = 0.5 * (we[:-1] + we[1:])
    return v2, g


def _profiles_for_lam(lam, v, dt):
    """q-gauge profiles A_j, Cp_j, betac_j for a vector of lam (float64)."""
    lam = np.asarray(lam, np.float64)
    v = v.astype(np.float64)
    v2 = v * v
    v_edge = 0.5 * (v[1:] + v[:-1])
    sqrt_eps = v_edge / np.sqrt(2.0)
    D = sqrt_eps[None, :] * lam[:, None]
    C = v_edge[None, :]
    w = C * DV / D
    delta = 1.0 / w - 1.0 / np.expm1(w)
    lo = C * delta - D / DV
    hi = C * (1.0 - delta) + D / DV
    w2 = v_edge ** 2
    w2lo, w2hi = w2 * lo, w2 * hi
    inv = 1.0 / (v2 * DV)
    Mn = lam.shape[0]
    z = np.zeros((Mn, 1))
    diagL = (np.concatenate([w2lo, z], -1) - np.concatenate([z, w2hi], -1)) * inv
    subL = np.concatenate([z, -w2lo], -1) * inv
    supL = np.concatenate([w2hi, z], -1) * inv
    k = float(dt) * NUEE_COEFF
    a = -k * subL
    b = 1.0 - k * diagL
    c = -k * supL
    alpha = np.zeros((Mn, NV))
    betac = np.zeros((Mn, NV))
    cp = np.zeros((Mn, NV))
    cprev = np.zeros(Mn)
    for j in range(NV):
        denom = b[:, j] - a[:, j] * cprev
        cprev = c[:, j] / denom
        cp[:, j] = cprev
        betac[:, j] = 1.0 / denom
        alpha[:, j] = -a[:, j] / denom
    A = np.zeros_like(alpha)
    A[:, 1:] = alpha[:, 1:] * betac[:, :-1] / betac[:, 1:]
    Cp = np.zeros_like(cp)
    Cp[:, :-1] = cp[:, :-1] * betac[:, 1:] / betac[:, :-1]
    return A, Cp, betac


def _split2(K):
    """2-term bf16 split: K ~= h + l to ~2^-16 relative."""
    h = K.astype(ml_dtypes.bfloat16)
    l = (K - h.astype(np.float64)).astype(ml_dtypes.bfloat16)
    return h, l


def _build_host_data(f0x, dt, v):
    """lam per row (f64) -> split-bf16 Chebyshev tables [KS, 3*NV] and the
    per-tile transposed split basis lhsT [KS, NX] (bf16)."""
    f64 = np.asarray(f0x, np.float64)
    v2, g = _host_weights(v)
    v4 = v2 * v2
    S2 = f64 @ v2
    S4 = f64 @ v4
    Sg = f64 @ g
    lam = Sg * S4 / (6.0 * DV * S2 * S2)
    lo, hi = float(lam.min()), float(lam.max())
    span = max(hi - lo, 1e-3 * max(abs(hi), 1e-30))
    lo -= 0.20 * span
    hi += 0.20 * span
    mid = 0.5 * (lo + hi)
    half = 0.5 * (hi - lo)

    kk = np.arange(M)
    xk = np.cos(np.pi * (kk + 0.5) / M)
    An, Cpn, Bn = _profiles_for_lam(mid + half * xk, v, dt)
    T = np.cos(np.outer(np.arange(M), np.pi * (kk + 0.5) / M))
    W = (2.0 / M) * T
    W[0, :] *= 0.5
    tabs = []
    for prof in (An, -Cpn[:, ::-1], Bn):
        Kc = W @ prof                                     # [M, NV] f64
        Kh, Kl = _split2(Kc)
        # rows [Kh; Kl; Kh] match lhsT rows [Ph; Ph; Pl]
        tabs.append(np.concatenate([Kh, Kl, Kh], axis=0))  # [KS, NV]
    ktab = np.ascontiguousarray(np.concatenate(tabs, axis=1))  # [KS, 3*NV]

    xi = ((lam - mid) / half).astype(np.float32)
    P = np.zeros((NX, M), np.float32)
    P[:, 0] = 1.0
    P[:, 1] = xi
    for m in range(2, M):
        P[:, m] = 2.0 * xi * P[:, m - 1] - P[:, m - 2]
    Ph = P.astype(ml_dtypes.bfloat16)
    Pl = (P - Ph.astype(np.float32)).astype(ml_dtypes.bfloat16)
    lhsT = np.concatenate([Ph, Ph, Pl], axis=1).T          # [KS, NX] bf16
    # replicate the tables at base partitions 0/32/64/96 so weights and rhs
    # can sit on any of the 4 bands (full-width DMA instead of 24-partition)
    kt128 = np.zeros((128, ktab.shape[1]), ktab.dtype)
    for t3 in range(3):
        kt128[32 * t3:32 * t3 + KS] = ktab
    return np.ascontiguousarray(lhsT), np.ascontiguousarray(kt128)


# ---------------------------------------------------------------- bass build

def build_program():
    nc = bacc.Bacc("TRN2", target_bir_lowering=False, debug=False)

    fin = nc.dram_tensor("fin", [ROWS, NV], F32, kind="ExternalInput").ap()
    lhsT = nc.dram_tensor("lhsT", [128, 768], BF16, kind="ExternalInput").ap()
    ktab = nc.dram_tensor("ktab", [128, 3 * NV], BF16,
                          kind="ExternalInput").ap()
    xout = nc.dram_tensor("xout", [ROWS, NV], F32, kind="ExternalOutput").ap()

    # tile-major views: [128 p, t, j] with 2KB-contiguous innermost runs
    fin_pt = fin.rearrange("(t p) j -> p t j", p=128)
    xout_pt = xout.rearrange("(t p) j -> p t j", p=128)

    # segments: 2-tile pairs throughout (scan-overhead amortization; the
    # carry self-clears at tile boundaries since A_0 = Cp_511 = 0).
    SEGS = ([(0, 1)] + [(a, a + 2) for a in range(1, NT - 1, 2)]
            + [(NT - 1, NT)])
    # x drain points: after which segment index to DMA which tile range
    DRAINS = {2: (0, 5), 4: (5, 9), 6: (9, 13), 7: (13, 15), 8: (15, 16)}

    with tile.TileContext(nc) as tc:
        with (
            tc.tile_pool(name="const", bufs=1) as cpool,
            tc.tile_pool(name="sol", bufs=2) as spool,
            tc.tile_pool(name="bcs", bufs=2) as bpool,
            # A and B tables share one ring (A(s) and B(s) alternate its two
            # 2-bank slots); C gets true double-buffering so the backward
            # scan never waits on table production.
            tc.tile_pool(name="psA", bufs=2, space="PSUM") as pA,
            tc.tile_pool(name="psC", bufs=2, space="PSUM") as pC,
        ):
            fall = cpool.tile([128, NT * NV], F32)
            xall = cpool.tile([128, NT * NV], F32)
            lt = cpool.tile([128, 768], BF16)
            kt = cpool.tile([128, 3 * NV], BF16)

            def band(T):
                g, t3 = T // 3, T % 3
                w = lt[32 * t3:32 * t3 + KS, 128 * g:128 * (g + 1)]
                return t3, w

            def krhs(T, c0, c1):
                t3 = T % 3
                return kt[32 * t3:32 * t3 + KS, c0:c1]

            def sb_rng(buf, a, b):
                return buf[:, a * NV:b * NV].rearrange(
                    "p (t j) -> p t j", j=NV)

            # DMA order chosen so tile 0's dependencies land first: the
            # A-table third of ktab and tile 0's basis columns are tiny and
            # gate the first matmul; f tile 0 follows so its transfer
            # overlaps it.  Transfers drain in global issue order.
            nc.sync.dma_start(sb_rng(fall, 0, 1), fin_pt[:, 0:1])
            nc.sync.dma_start(kt[:, 0:NV], ktab[:, 0:NV])
            nc.scalar.dma_start(lt[:], lhsT)
            nc.scalar.dma_start(kt[:, NV:3 * NV], ktab[:, NV:3 * NV])
            nc.sync.dma_start(sb_rng(fall, 1, 3), fin_pt[:, 1:3])
            for a in range(3, NT, 4):
                b = min(a + 4, NT)
                nc.sync.dma_start(sb_rng(fall, a, b), fin_pt[:, a:b])

            def emit_A(oA, a, b):
                for i in range(b - a):
                    _, wt = band(a + i)
                    nc.tensor.matmul(oA[:, i * NV:(i + 1) * NV], wt,
                                     krhs(a + i, 0, NV),
                                     start=True, stop=True)

            # pre-fill both ring slots with the first two segments' A tables
            oA_pre = []
            for (a, b) in SEGS[:2]:
                oA = pA.tile([128, 2 * NV], F32, tag="oAB")
                emit_A(oA, a, b)
                oA_pre.append(oA)

            for si, (a, b) in enumerate(SEGS):
                W = b - a
                fw = fall[:, a * NV:b * NV]
                xw = xall[:, a * NV:b * NV]
                if si < 2:
                    oA = oA_pre[si]
                else:
                    oA = pA.tile([128, 2 * NV], F32, tag="oAB")
                    emit_A(oA, a, b)
                oC = pC.tile([128, 2 * NV], F32, tag="oC")
                oB = pA.tile([128, 2 * NV], F32, tag="oAB")
                # bwd tables reversed-within-tile AND tile-order-swapped
                # (the reversed stream hits the last tile first)
                for i in range(W):
                    _, wt = band(a + i)
                    nc.tensor.matmul(oC[:, (W - 1 - i) * NV:(W - i) * NV], wt,
                                     krhs(a + i, NV, 2 * NV),
                                     start=True, stop=True)
                    nc.tensor.matmul(oB[:, i * NV:(i + 1) * NV], wt,
                                     krhs(a + i, 2 * NV, 3 * NV),
                                     start=True, stop=True)
                qw = spool.tile([128, 2 * NV], F32, tag="qw")
                nc.vector.tensor_tensor_scan(
                    out=qw[:, 0:W * NV], data0=oA[:, 0:W * NV], data1=fw,
                    initial=0.0, op0=ALU.mult, op1=ALU.add)
                rw = spool.tile([128, 2 * NV], F32, tag="rw")
                nc.vector.tensor_tensor_scan(
                    out=rw[:, 0:W * NV][:, ::-1], data0=oC[:, 0:W * NV],
                    data1=qw[:, 0:W * NV][:, ::-1],
                    initial=0.0, op0=ALU.mult, op1=ALU.add)
                if si < len(SEGS) - 1:
                    bcs = bpool.tile([128, 2 * NV], F32, tag="bcs")
                    nc.scalar.copy(bcs[:, 0:W * NV], oB[:, 0:W * NV])
                    nc.gpsimd.tensor_tensor(xw, bcs[:, 0:W * NV],
                                            rw[:, 0:W * NV], ALU.mult)
                else:
                    # last (solo) segment: VectorE is free after its final
                    # scan and reads the table from PSUM -- shortest tail.
                    nc.vector.tensor_tensor(xw, oB[:, 0:W * NV],
                                            rw[:, 0:W * NV], ALU.mult)
                if si in DRAINS:
                    da, db = DRAINS[si]
                    nc.sync.dma_start(xout_pt[:, da:db], sb_rng(xall, da, db))

    nc.compile()
    return nc


_PROGRAM_CACHE = {}


def _get_program():
    if "prog" not in _PROGRAM_CACHE:
        _PROGRAM_CACHE["prog"] = build_program()
    return _PROGRAM_CACHE["prog"]


def make_in_maps(f0x, dt, v):
    f0x = np.ascontiguousarray(np.asarray(f0x, np.float32))
    v = np.asarray(v, np.float32)
    lhsT, ktab = _build_host_data(f0x, float(dt), v)
    in_maps = []
    for c in range(N_CORES):
        ltc = lhsT[:, c * ROWS:(c + 1) * ROWS]       # [KS, 2048]
        lt128 = np.zeros((128, 768), ltc.dtype)
        for T in range(NT):
            g, t3 = T // 3, T % 3
            lt128[32 * t3:32 * t3 + KS, 128 * g:128 * (g + 1)] = \
                ltc[:, T * 128:(T + 1) * 128]
        in_maps.append({
            "fin": np.ascontiguousarray(f0x[c * ROWS:(c + 1) * ROWS]),
            "lhsT": np.ascontiguousarray(lt128),
            "ktab": ktab,
        })
    return in_maps


def kernel(nu, f0x, dt, v):
    import os
    import time
    nc = _get_program()
    in_maps = make_in_maps(f0x, dt, v)
    trace = bool(os.environ.get("KERNEL_TRACE"))
    res = None
    last_exc = None
    for attempt in range(3):
        try:
            res = run_bass_kernel_spmd(nc, in_maps,
                                       core_ids=list(range(N_CORES)),
                                       trace=trace)
            break
        except Exception as e:   # transient device wedges have been observed
            last_exc = e
            time.sleep(5.0 * (attempt + 1))
    if res is None:
        raise last_exc
    if trace:
        kernel.last_results = res
    out = np.concatenate([r["xout"] for r in res.results], axis=0)
    return out.astype(np.float32)


# revision 31
# speedup vs baseline: 1.0185x; 1.0185x over previous
"""Trainium2 Bass kernel for nn_F0Collisions: batched Chang-Cooper implicit
Fokker-Planck solve, 16384 x 512, data-parallel over rows across 8 cores.

Method (v3.1): each row's tridiagonal system depends on the row only through
one scalar lam (the 3-step beta fixed point collapses to beta = 1/T_f on
this grid).  The host computes lam per row exactly (float64 moments -- the
same moments it must compute anyway to calibrate the Chebyshev interval)
and ships, per 128-row tile, the transposed 2-term-bf16-split Chebyshev
basis lhsT [24, 128].  The Thomas solve is re-gauged ("q-gauge") so the
forward scan consumes f directly (no premultiply):

    A_j  = alpha_j * betac_{j-1}/betac_j      (A_0 = 0)
    q_j  = A_j q_{j-1} + f_j                  (forward scan)
    Cp_j = cp_j * betac_{j+1}/betac_j         (Cp_511 = 0)
    r_j  = q_j - Cp_j r_{j+1}                 (backward scan)
    x_j  = betac_j * r_j                      (one final multiply)

Device work per 128x512 tile: 3 PE matmuls interpolate the A / -Cp / betac
profiles into PSUM; VectorE runs the two scans reading the tables straight
from PSUM (1-port SBUF -> immune to Pool port contention); ScalarE copies
betac PSUM->SBUF; the Pool engine does the final elementwise multiply.
Tiles are processed in PAIRS: since A_0 = 0 and Cp_511 = 0 exactly (table
columns are identically zero), the scan carry self-clears at tile
boundaries, so each scan instruction covers two tiles [128, 1024] with the
pair's tables adjacent in a 2-bank PSUM tile (bwd pair halves swapped to
match the reversed stream).  VectorE does nothing but the 16 scans -- the
roofline for this kernel.  The last pair's final multiply runs on VectorE
after its scans (shortest tail), all others on Pool.
"""

import numpy as np
import ml_dtypes

import concourse.bass as bass
import concourse.mybir as mybir
import concourse.tile as tile
from concourse import bacc
from concourse.bass_utils import run_bass_kernel_spmd

NX, NV = 16384, 512
N_CORES = 8
ROWS = NX // N_CORES          # rows per core
NT = ROWS // 128              # 128-row tiles per core
NP = NT // 2                  # tile pairs
DV = 8.0 / NV
NUEE_COEFF = 2.221e-7
M = 8                         # Chebyshev terms
KS = 3 * M                    # [Ph; Ph; Pl] x [Kh; Kl; Kh] split contraction

F32 = mybir.dt.float32
BF16 = mybir.dt.bfloat16
ALU = mybir.AluOpType
AFT = mybir.ActivationFunctionType


# ---------------------------------------------------------------- host math

def _host_weights(v):
    """v2 and g weight vectors (float64): S2 = sum f*v2, Sg = sum f*g."""
    v = v.astype(np.float64)
    v2 = v * v
    we = (0.5 * (v[1:] + v[:-1])) ** 2 * DV / np.sqrt(2.0)   # sqrt_eps * d_eps
    g = np.empty(NV)
    g[0] = 0.5 * we[0]
    g[-1] = 0.5 * we[-1]
    g[1:-1] = 0.5 * (we[:-1] + we[1:])
    return v2, g


def _profiles_for_lam(lam, v, dt):
    """q-gauge profiles A_j, Cp_j, betac_j for a vector of lam (float64)."""
    lam = np.asarray(lam, np.float64)
    v = v.astype(np.float64)
    v2 = v * v
    v_edge = 0.5 * (v[1:] + v[:-1])
    sqrt_eps = v_edge / np.sqrt(2.0)
    D = sqrt_eps[None, :] * lam[:, None]
    C = v_edge[None, :]
    w = C * DV / D
    delta = 1.0 / w - 1.0 / np.expm1(w)
    lo = C * delta - D / DV
    hi = C * (1.0 - delta) + D / DV
    w2 = v_edge ** 2
    w2lo, w2hi = w2 * lo, w2 * hi
    inv = 1.0 / (v2 * DV)
    Mn = lam.shape[0]
    z = np.zeros((Mn, 1))
    diagL = (np.concatenate([w2lo, z], -1) - np.concatenate([z, w2hi], -1)) * inv
    subL = np.concatenate([z, -w2lo], -1) * inv
    supL = np.concatenate([w2hi, z], -1) * inv
    k = float(dt) * NUEE_COEFF
    a = -k * subL
    b = 1.0 - k * diagL
    c = -k * supL
    alpha = np.zeros((Mn, NV))
    betac = np.zeros((Mn, NV))
    cp = np.zeros((Mn, NV))
    cprev = np.zeros(Mn)
    for j in range(NV):
        denom = b[:, j] - a[:, j] * cprev
        cprev = c[:, j] / denom
        cp[:, j] = cprev
        betac[:, j] = 1.0 / denom
        alpha[:, j] = -a[:, j] / denom
    A = np.zeros_like(alpha)
    A[:, 1:] = alpha[:, 1:] * betac[:, :-1] / betac[:, 1:]
    Cp = np.zeros_like(cp)
    Cp[:, :-1] = cp[:, :-1] * betac[:, 1:] / betac[:, :-1]
    return A, Cp, betac


def _split2(K):
    """2-term bf16 split: K ~= h + l to ~2^-16 relative."""
    h = K.astype(ml_dtypes.bfloat16)
    l = (K - h.astype(np.float64)).astype(ml_dtypes.bfloat16)
    return h, l


def _build_host_data(f0x, dt, v):
    """lam per row (f64) -> split-bf16 Chebyshev tables [KS, 3*NV] and the
    per-tile transposed split basis lhsT [KS, NX] (bf16)."""
    f64 = np.asarray(f0x, np.float64)
    v2, g = _host_weights(v)
    v4 = v2 * v2
    S2 = f64 @ v2
    S4 = f64 @ v4
    Sg = f64 @ g
    lam = Sg * S4 / (6.0 * DV * S2 * S2)
    lo, hi = float(lam.min()), float(lam.max())
    span = max(hi - lo, 1e-3 * max(abs(hi), 1e-30))
    lo -= 0.20 * span
    hi += 0.20 * span
    mid = 0.5 * (lo + hi)
    half = 0.5 * (hi - lo)

    kk = np.arange(M)
    xk = np.cos(np.pi * (kk + 0.5) / M)
    An, Cpn, Bn = _profiles_for_lam(mid + half * xk, v, dt)
    T = np.cos(np.outer(np.arange(M), np.pi * (kk + 0.5) / M))
    W = (2.0 / M) * T
    W[0, :] *= 0.5
    tabs = []
    for prof in (An, -Cpn[:, ::-1], Bn):
        Kc = W @ prof                                     # [M, NV] f64
        Kh, Kl = _split2(Kc)
        # rows [Kh; Kl; Kh] match lhsT rows [Ph; Ph; Pl]
        tabs.append(np.concatenate([Kh, Kl, Kh], axis=0))  # [KS, NV]
    ktab = np.ascontiguousarray(np.concatenate(tabs, axis=1))  # [KS, 3*NV]

    xi = ((lam - mid) / half).astype(np.float32)
    P = np.zeros((NX, M), np.float32)
    P[:, 0] = 1.0
    P[:, 1] = xi
    for m in range(2, M):
        P[:, m] = 2.0 * xi * P[:, m - 1] - P[:, m - 2]
    Ph = P.astype(ml_dtypes.bfloat16)
    Pl = (P - Ph.astype(np.float32)).astype(ml_dtypes.bfloat16)
    lhsT = np.concatenate([Ph, Ph, Pl], axis=1).T          # [KS, NX] bf16
    return np.ascontiguousarray(lhsT), ktab


# ---------------------------------------------------------------- bass build

def build_program():
    nc = bacc.Bacc("TRN2", target_bir_lowering=False, debug=False)

    fin = nc.dram_tensor("fin", [ROWS, NV], F32, kind="ExternalInput").ap()
    lhsT = nc.dram_tensor("lhsT", [KS, ROWS], BF16, kind="ExternalInput").ap()
    ktab = nc.dram_tensor("ktab", [KS, 3 * NV], BF16, kind="ExternalInput").ap()
    xout = nc.dram_tensor("xout", [ROWS, NV], F32, kind="ExternalOutput").ap()

    # tile-major views: [128 p, t, j] with 2KB-contiguous innermost runs
    fin_pt = fin.rearrange("(t p) j -> p t j", p=128)
    xout_pt = xout.rearrange("(t p) j -> p t j", p=128)

    # segments: 2-tile pairs throughout (scan-overhead amortization; the
    # carry self-clears at tile boundaries since A_0 = Cp_511 = 0).
    SEGS = ([(0, 1)] + [(a, a + 2) for a in range(1, NT - 1, 2)]
            + [(NT - 1, NT)])
    # x drain points: after which segment index to DMA which tile range
    DRAINS = {2: (0, 5), 4: (5, 9), 6: (9, 13), 7: (13, 15), 8: (15, 16)}

    with tile.TileContext(nc) as tc:
        with (
            tc.tile_pool(name="const", bufs=1) as cpool,
            tc.tile_pool(name="sol", bufs=2) as spool,
            tc.tile_pool(name="bcs", bufs=2) as bpool,
            # A and B tables share one ring (A(s) and B(s) alternate its two
            # 2-bank slots); C gets true double-buffering so the backward
            # scan never waits on table production.
            tc.tile_pool(name="psA", bufs=2, space="PSUM") as pA,
            tc.tile_pool(name="psC", bufs=2, space="PSUM") as pC,
        ):
            fall = cpool.tile([128, NT * NV], F32)
            xall = cpool.tile([128, NT * NV], F32)
            lt = cpool.tile([KS, ROWS], BF16)
            kt = cpool.tile([KS, 3 * NV], BF16)

            def sb_rng(buf, a, b):
                return buf[:, a * NV:b * NV].rearrange(
                    "p (t j) -> p t j", j=NV)

            # DMA order chosen so tile 0's dependencies land first: the
            # A-table third of ktab and tile 0's basis columns are tiny and
            # gate the first matmul; f tile 0 follows so its transfer
            # overlaps it.  Transfers drain in global issue order.
            nc.sync.dma_start(sb_rng(fall, 0, 1), fin_pt[:, 0:1])
            nc.sync.dma_start(kt[:, 0:NV], ktab[:, 0:NV])
            nc.scalar.dma_start(lt[:, 0:384], lhsT[:, 0:384])
            nc.scalar.dma_start(kt[:, NV:3 * NV], ktab[:, NV:3 * NV])
            nc.scalar.dma_start(lt[:, 384:1152], lhsT[:, 384:1152])
            nc.scalar.dma_start(lt[:, 1152:ROWS], lhsT[:, 1152:ROWS])
            nc.sync.dma_start(sb_rng(fall, 1, 3), fin_pt[:, 1:3])
            for a in range(3, NT, 4):
                b = min(a + 4, NT)
                nc.sync.dma_start(sb_rng(fall, a, b), fin_pt[:, a:b])

            def emit_A(oA, a, b):
                for i in range(b - a):
                    wt = lt[:, (a + i) * 128:(a + i + 1) * 128]
                    nc.tensor.matmul(oA[:, i * NV:(i + 1) * NV], wt,
                                     kt[:, 0:NV], start=True, stop=True)

            # pre-fill both ring slots with the first two segments' A tables
            oA_pre = []
            for (a, b) in SEGS[:2]:
                oA = pA.tile([128, 2 * NV], F32, tag="oAB")
                emit_A(oA, a, b)
                oA_pre.append(oA)

            for si, (a, b) in enumerate(SEGS):
                W = b - a
                fw = fall[:, a * NV:b * NV]
                xw = xall[:, a * NV:b * NV]
                if si < 2:
                    oA = oA_pre[si]
                else:
                    oA = pA.tile([128, 2 * NV], F32, tag="oAB")
                    emit_A(oA, a, b)
                oC = pC.tile([128, 2 * NV], F32, tag="oC")
                oB = pA.tile([128, 2 * NV], F32, tag="oAB")
                # bwd tables reversed-within-tile AND tile-order-swapped
                # (the reversed stream hits the last tile first)
                for i in range(W):
                    wt = lt[:, (a + i) * 128:(a + i + 1) * 128]
                    nc.tensor.matmul(oC[:, (W - 1 - i) * NV:(W - i) * NV], wt,
                                     kt[:, NV:2 * NV], start=True, stop=True)
                    nc.tensor.matmul(oB[:, i * NV:(i + 1) * NV], wt,
                                     kt[:, 2 * NV:3 * NV], start=True,
                                     stop=True)
                qw = spool.tile([128, 2 * NV], F32, tag="qw")
                nc.vector.tensor_tensor_scan(
                    out=qw[:, 0:W * NV], data0=oA[:, 0:W * NV], data1=fw,
                    initial=0.0, op0=ALU.mult, op1=ALU.add)
                rw = spool.tile([128, 2 * NV], F32, tag="rw")
                nc.vector.tensor_tensor_scan(
                    out=rw[:, 0:W * NV][:, ::-1], data0=oC[:, 0:W * NV],
                    data1=qw[:, 0:W * NV][:, ::-1],
                    initial=0.0, op0=ALU.mult, op1=ALU.add)
                if si < len(SEGS) - 1:
                    bcs = bpool.tile([128, 2 * NV], F32, tag="bcs")
                    nc.scalar.copy(bcs[:, 0:W * NV], oB[:, 0:W * NV])
                    nc.gpsimd.tensor_tensor(xw, bcs[:, 0:W * NV],
                                            rw[:, 0:W * NV], ALU.mult)
                else:
                    # last (solo) segment: VectorE is free after its final
                    # scan and reads the table from PSUM -- shortest tail.
                    nc.vector.tensor_tensor(xw, oB[:, 0:W * NV],
                                            rw[:, 0:W * NV], ALU.mult)
                if si in DRAINS:
                    da, db = DRAINS[si]
                    nc.sync.dma_start(xout_pt[:, da:db], sb_rng(xall, da, db))

    nc.compile()
    return nc


_PROGRAM_CACHE = {}


def _get_program():
    if "prog" not in _PROGRAM_CACHE:
        _PROGRAM_CACHE["prog"] = build_program()
    return _PROGRAM_CACHE["prog"]


def make_in_maps(f0x, dt, v):
    f0x = np.ascontiguousarray(np.asarray(f0x, np.float32))
    v = np.asarray(v, np.float32)
    lhsT, ktab = _build_host_data(f0x, float(dt), v)
    in_maps = []
    for c in range(N_CORES):
        in_maps.append({
            "fin": np.ascontiguousarray(f0x[c * ROWS:(c + 1) * ROWS]),
            "lhsT": np.ascontiguousarray(lhsT[:, c * ROWS:(c + 1) * ROWS]),
            "ktab": ktab,
        })
    return in_maps


def kernel(nu, f0x, dt, v):
    import os
    import time
    nc = _get_program()
    in_maps = make_in_maps(f0x, dt, v)
    trace = bool(os.environ.get("KERNEL_TRACE"))
    res = None
    last_exc = None
    for attempt in range(3):
        try:
            res = run_bass_kernel_spmd(nc, in_maps,
                                       core_ids=list(range(N_CORES)),
                                       trace=trace)
            break
        except Exception as e:   # transient device wedges have been observed
            last_exc = e
            time.sleep(5.0 * (attempt + 1))
    if res is None:
        raise last_exc
    if trace:
        kernel.last_results = res
    out = np.concatenate([r["xout"] for r in res.results], axis=0)
    return out.astype(np.float32)
